# revision 103
# baseline (speedup 1.0000x reference)
"""Trainium2 Bass kernel for the CNN-MAD per-class DTW transport cost.

Math (reference):
  mat_cost[n, j] = C1[n] + C2[c_n, j] - 2*C3[n, j],  c_n = classes[n]
    C1[n]    = sum_t rowsum[c_n, t] * ||X[n,t,:]||^2
    C2[c, j] = sum_p colsum[c, p] * ||Y[j,p,:]||^2
    C3[n, j] = sum_{p,d} (sum_t pi[c_n,t,p] X[n,t,d]) * Y[j,p,d]

Sharding 4x2: core k = (g, h) with g = k>>1 (class group: classes 2g, 2g+1,
each padded to 144 sample slots) and h = k&1 (Y half, 512 rows). The host
only regroups / transposes / dtype-casts; all arithmetic is on device.

Precision: inputs cast to fp8 e4m3 on host (X, Y absmax ~5.4; pi is 0/1 so
exact). Heavy matmuls run fp8 DoubleRow (K=256/instr) into f32 PSUM. The
large C1/C2 terms ride fp16 paths (C1 via a K=1 ones matmul into PSUM, C2
transposed [j, c] added as a per-partition bias during the fp16 output
evacuation). End-to-end rel err ~1e-3 vs the 2e-2 gate.

Device layout per core (C3 contraction k=(pt,d,pp), t=(tt,tp)):
  pis [tp 128, (c 2, tt 2, p 256) | (c 2, pt 2, t 256)]  fp8 (pi and pi^T)
  xk  [tp 128, d 8, tt 2, n 288]  fp8   xk[tp,d,tt,n] = X[n, tt*128+tp, d]
  yt  [pp 128, kc 16, j 512]      fp8   kc=(pt,d): yt = Y[j, pt*128+pp, d]
  crps: colsum^T / rowsum^T via 8 DoubleRow ones-matmuls (one PSUM bank)
  XW:  per (pt,d) granule, 2 class DoubleRows; evac * -2 -> xwt fp8
  xsq/ysq: elementwise fp8 squares split across ACT/DVE/Pool
  C1:  DoubleRow rw8.T @ xsq -> psum [2, 288] -> fp16 + per-class select
       -> c1row [1, 288]; added into each C3 psum by a K=1 ones matmul
  C2t: DoubleRow ysq.T @ cs2 -> psum [j 128, c 2] per jt (transposed C2)
  C3:  kc-pair DoubleRow yt.T @ xwt into 4 psum banks [j 128, n 288]
  out: fp16 evac with per-partition bias C2t[j, c], two DMAs out
"""

import sys

sys.path.insert(0, "/opt/trn_rl_repo")

import numpy as np
import ml_dtypes

N, NY, T, TP, D, C = 1024, 1024, 256, 256, 8, 8
NCORES = 8
G, H = 4, 2          # class groups x Y halves
CPC = 144            # per-class sample capacity (max count is 144)
CAP = 2 * CPC        # 288 sample columns per core
NYH = NY // H        # 512
KC = 16              # 128-row contraction chunks of C3, kc = (pt, d)
JT = NYH // 128      # 4 output row tiles

FP8 = ml_dtypes.float8_e4m3

_cache = {}


def _build():
    import concourse.bacc as bacc
    import concourse.mybir as mybir
    import concourse.tile as tile

    f8 = mybir.dt.float8e4
    f16 = mybir.dt.float16
    f32 = mybir.dt.float32
    DR = mybir.MatmulPerfMode.DoubleRow
    Ident = mybir.ActivationFunctionType.Identity
    nc = bacc.Bacc("TRN2", target_bir_lowering=False, debug=False, num_devices=NCORES)

    pis_d = nc.dram_tensor("pis", [128, 2 * 2 * 2 * TP], f8, kind="ExternalInput")
    xk_d = nc.dram_tensor("xk", [128, KC * CAP], f8, kind="ExternalInput")
    yt_d = nc.dram_tensor("yt", [128, KC * NYH], f8, kind="ExternalInput")
    out_d = nc.dram_tensor("outp", [NYH, CAP], f16, kind="ExternalOutput")

    with tile.TileContext(nc) as tc:
        with (
            tc.tile_pool(name="const", bufs=1) as pc,
            tc.tile_pool(name="xin", bufs=1) as px,
            tc.tile_pool(name="yin", bufs=1) as py,
            tc.tile_pool(name="psA", bufs=7, space="PSUM") as psA,
            tc.tile_pool(name="psB", bufs=1, space="PSUM") as psB,
        ):
            # ---- input DMAs on the SP HWDGE queue ----
            pis = pc.tile([128, 2, 2, 2, TP], f8, tag="pis")
            pisv = pis_d.rearrange("l (w c u p) -> l w c u p", w=2, c=2, u=2)
            pi = pis[:, 0, :, :, :]    # [tp, c, tt, p]
            piT = pis[:, 1, :, :, :]   # [pp, c, pt, t]

            xk = px.tile([128, D, 2, CAP], f8, tag="xk")
            xkv = xk_d.rearrange("l (d u n) -> l d u n", d=D, u=2)
            yt = py.tile([128, KC, NYH], f8, tag="yt")
            ytv = yt_d.rearrange("l (k j) -> l k j", k=KC)
            # pi first alone (crps/XW need it); piT only feeds the late
            # rowsum -> C1 path, so it rides after xk half 1
            nc.sync.dma_start(pis[:, 0, :, :, :], pisv[:, 0, :, :, :])
            nc.sync.dma_start(yt[:, 0:4, :], ytv[:, 0:4, :])
            nc.sync.dma_start(yt[:, 4:8, :], ytv[:, 4:8, :])
            nc.sync.dma_start(xk[:, 0:4, :, :], xkv[:, 0:4, :, :])
            nc.sync.dma_start(pis[:, 1, :, :, :], pisv[:, 1, :, :, :])
            nc.sync.dma_start(yt[:, 8:12, :], ytv[:, 8:12, :])
            nc.sync.dma_start(xk[:, 4:8, :, :], xkv[:, 4:8, :, :])
            nc.sync.dma_start(yt[:, 12:16, :], ytv[:, 12:16, :])

            # ---- small constants (Pool; everything tiny and early) ----
            # DoubleRow operands need a k-group stride that is a multiple of
            # 16 bytes (s3_lw_dual_fp8_restrictions), hence the padded tiles.
            ones8p = pc.tile([128, 2, 16], f8, tag="ones8p")
            nc.gpsimd.memset(ones8p[:], 1.0)
            ones8 = ones8p[:, :, 0:1]
            ones16 = pc.tile([1, 128], f16, tag="ones16")
            nc.gpsimd.memset(ones16[:], 1.0)

            # ---- colsum^T (c,pt) and rowsum^T (c,tt) via ones DoubleRows ----
            # cols 0-7: colsum/rowsum sums; cols 8-295 host the C1 rows
            # later (the bank is fully consumed by cs2/rw8p by then, so
            # C1's start=True bank-wipe is harmless)
            crps = psB.tile([128, 16 + CAP], f32, tag="psB", name="crps")
            for c in range(2):
                for pt in range(2):
                    nc.tensor.matmul(
                        crps[:, 2 * c + pt : 2 * c + pt + 1],
                        pi[:, c, :, pt * 128 : (pt + 1) * 128],
                        ones8,
                        start=(c == 0 and pt == 0), stop=True, perf_mode=DR,
                        skip_group_check=True,
                    )
            for c in range(2):
                for tt in range(2):
                    nc.tensor.matmul(
                        crps[:, 4 + 2 * c + tt : 5 + 2 * c + tt],
                        piT[:, c, :, tt * 128 : (tt + 1) * 128],
                        ones8,
                        start=False, stop=True, perf_mode=DR,
                        skip_group_check=True,
                    )
            cs2 = pc.tile([128, 2, 2], f8, tag="cs2")      # [pp, pt, c]
            csv = crps[:, 0:4].rearrange("l (c pt) -> l pt c", c=2)
            nc.vector.tensor_copy(cs2[:], csv)
            rw8p = pc.tile([128, 2, 16], f8, tag="rw8p")   # [tp, tt, c pad]
            nc.vector.tensor_copy(rw8p[:, :, 0:2], crps[:, 4:8].rearrange("l (c u) -> l u c", c=2))

            # one xwt tile per kc PAIR, each written by a single engine:
            # C3's pair-r read then depends only on its own pair's evac
            # (a shared tile serialized all C3 behind the last evacuation)
            xwp = [
                px.tile([128, 2, CAP], f8, tag=f"xwp{i}", name=f"xwp{i}")
                for i in range(8)
            ]
            xsq = px.tile([128, D, 2, CAP], f8, tag="xsq")
            # one ysq tile per writer engine: a shared tile serializes the
            # writers through false whole-tile WAW dependencies
            ysqD = py.tile([128, 2, NYH], f8, tag="ysqD")    # kc 0-1   (DVE)
            ysqB = py.tile([128, 2, NYH], f8, tag="ysqB")    # kc 2-3   (ACT)
            ysqP = py.tile([128, 8, NYH], f8, tag="ysqP")    # kc 4-11  (Pool)
            ysqE = py.tile([128, 2, NYH], f8, tag="ysqE")    # kc 12-13 (DVE)
            ysqA = py.tile([128, 2, NYH], f8, tag="ysqA")    # kc 14-15 (ACT)

            def ysq_sl(kc, js):
                if kc < 2:
                    return ysqD[:, kc, js]
                if kc < 4:
                    return ysqB[:, kc - 2, js]
                if kc < 12:
                    return ysqP[:, kc - 4, js]
                if kc < 14:
                    return ysqE[:, kc - 12, js]
                return ysqA[:, kc - 14, js]

            def xw_granule(pt, d, evac):
                g = psA.tile([128, CAP], f32, tag="psA", name=f"xw{pt}_{d}")
                for cl in range(2):
                    nc.tensor.matmul(
                        g[:, cl * CPC : (cl + 1) * CPC],
                        pi[:, cl, :, pt * 128 : (pt + 1) * 128],
                        xk[:, d, :, cl * CPC : (cl + 1) * CPC],
                        start=True, stop=True, perf_mode=DR,
                        skip_group_check=True,
                    )
                evac(xwp[pt * 4 + d // 2][:, d % 2, :], g[:], -2.0)

            dve_m, act_m, pool_m = (nc.vector.tensor_scalar_mul, nc.scalar.mul,
                                    nc.gpsimd.tensor_scalar_mul)

            # Pool (GPSIMD) cannot touch PSUM on real HW, and only plain
            # tensor_tensor ops compile for it: it streams mid ysq chunks.
            def pool_sq(dst, src):
                nc.gpsimd.tensor_mul(dst, src, src)

            # early ysq on the fast engines while they wait for xk; Pool
            # streams the mid chunks kc 4-11
            nc.vector.tensor_mul(ysqD[:], yt[:, 0:2, :], yt[:, 0:2, :])
            nc.scalar.square(ysqB[:], yt[:, 2:4, :])
            pool_sq(ysqP[:, 0:4, :], yt[:, 4:8, :])
            pool_sq(ysqP[:, 4:8, :], yt[:, 8:12, :])

            # ---- xk half 1 (d 0-3): XW kc 0-3, 8-11; xsq quad on DVE ----
            # ACT has exec-queue depth 0 (strictly in-order), so its queue
            # must be emitted in data-arrival order: the (1, d2-3) granule
            # evacs (ready late) go AFTER xsq_b below, or they block it.
            for d in range(4):
                xw_granule(0, d, (dve_m, dve_m, act_m, act_m)[d])
            for d in range(2):
                xw_granule(1, d, dve_m)
            nc.vector.tensor_mul(xsq[:, 0:2, :, :], xk[:, 0:2, :, :], xk[:, 0:2, :, :])
            nc.vector.tensor_mul(xsq[:, 2:4, :, :], xk[:, 2:4, :, :], xk[:, 2:4, :, :])

            # ---- xk half 2 (d 4-7): xsq quad on ACT first (C1 chain) ----
            nc.scalar.square(xsq[:, 4:8, :, :], xk[:, 4:8, :, :])
            xw_granule(1, 2, act_m)
            xw_granule(1, 3, act_m)
            # final ysq chunks slot in here: the data lands about when the
            # engines reach this point, and C2 can close early
            nc.vector.tensor_mul(ysqE[:, 0:1, :], yt[:, 12:13, :], yt[:, 12:13, :])
            nc.vector.tensor_mul(ysqE[:, 1:2, :], yt[:, 13:14, :], yt[:, 13:14, :])
            nc.scalar.square(ysqA[:], yt[:, 14:16, :])
            for d in range(4, 8):
                xw_granule(0, d, (dve_m, dve_m, act_m, act_m)[d - 4])
            for d in range(4, 8):
                xw_granule(1, d, (dve_m, dve_m, act_m, act_m)[d - 4])

            # ---- C1 per class: DoubleRows -> two [1, CAP] psum rows ----
            # Each class's C1 only needs its own 144 columns, so both rows
            # fit disjoint regions of the crps bank (no psA slot: the psA
            # rotation edge made C1 gate the augs). One start/stop bracket
            # for the shared bank, one copy out.
            for d in range(D):
                for cl in range(2):
                    nc.tensor.matmul(
                        crps[0:1, 8 + cl * CPC : 8 + (cl + 1) * CPC],
                        rw8p[:, :, cl : cl + 1],
                        xsq[:, d, :, cl * CPC : (cl + 1) * CPC],
                        start=False,
                        stop=(d == D - 1 and cl == 1),
                        perf_mode=DR, skip_group_check=True,
                    )
            c1row = pc.tile([1, CAP], f16, tag="c1row")
            nc.vector.tensor_copy(c1row[0:1, :], crps[0:1, 8 : 8 + CAP])


            # ---- streaming: remaining ysq + C3/C2t per kc pair ----
            c2ps = crps[:, 8 + CAP : 16 + CAP].rearrange(
                "l (jt c) -> l jt c", c=2
            )
            c3ps = [
                psA.tile([128, CAP], f32, tag="psA", name=f"c3_{jt}")
                for jt in range(JT)
            ]

            def c3_sl(jt):
                return c3ps[jt][:]

            def c3_r(r, start):
                for jt in range(JT):
                    nc.tensor.matmul(
                        c3_sl(jt),
                        yt[:, 2 * r : 2 * r + 2, jt * 128 : (jt + 1) * 128],
                        xwp[r][:],
                        start=start, stop=False, perf_mode=DR,
                        skip_group_check=True,
                    )

            def c2_r(r, start, stop):
                # Plain matmuls (2-wide-ifmap DoubleRow miscomputes on HW).
                # One start/stop bracket for the whole bank: a start=True
                # clears has-written for the entire 2KB PSUM bank, so only
                # the very first matmul of the shared bank may carry it.
                pt = r // 4
                for kc in (2 * r, 2 * r + 1):
                    for jt in range(JT):
                        nc.tensor.matmul(
                            c2ps[:, jt, :],
                            ysq_sl(kc, slice(jt * 128, (jt + 1) * 128)),
                            cs2[:, pt, :],
                            start=False,
                            stop=(stop and kc == 2 * r + 1 and jt == JT - 1),
                            skip_group_check=True,
                        )

            c3_r(0, True); c2_r(0, True, False)
            c3_r(1, False); c2_r(1, False, False)
            c3_r(2, False); c2_r(2, False, False)
            c3_r(3, False); c2_r(3, False, False)
            c3_r(4, False); c2_r(4, False, False)
            c3_r(5, False); c2_r(5, False, False)
            c3_r(6, False); c2_r(6, False, False)
            # close C2 first: it only needs ysq, so the C2 bias is ready
            # long before the C3 psums close
            c2_r(7, False, True)
            c2sb = pc.tile([128, JT, 2], f32, tag="c2sb")
            nc.scalar.copy(c2sb[:], c2ps[:])
            # close each C3 group with one K=1 fp16 C1 augmentation right
            # after its last kc pair
            for jt in range(JT):
                nc.tensor.matmul(
                    c3_sl(jt),
                    yt[:, 14:16, jt * 128 : (jt + 1) * 128],
                    xwp[7][:],
                    start=False, stop=False, perf_mode=DR,
                    skip_group_check=True,
                )
                nc.tensor.matmul(
                    c3_sl(jt), ones16[:], c1row[:],
                    start=False, stop=True, skip_group_check=True,
                )

            # ---- out: fp16 evac with per-partition C2 bias, 2 DMAs ----
            osb = py.tile([128, JT, CAP], f16, tag="osb")
            odv = out_d.rearrange("(jt l) n -> l jt n", l=128)

            def bias_evac(eng, jt, cl):
                dst = osb[:, jt, cl * CPC : (cl + 1) * CPC]
                src = c3_sl(jt)[:, cl * CPC : (cl + 1) * CPC]
                b = c2sb[:, jt, cl : cl + 1]
                if eng is nc.scalar:
                    eng.activation(dst, src, Ident, bias=b)
                else:
                    eng.tensor_scalar_add(dst, src, b)

            bias_evac(nc.vector, 0, 0); bias_evac(nc.scalar, 0, 1)
            bias_evac(nc.vector, 1, 0); bias_evac(nc.scalar, 1, 1)
            nc.sync.dma_start(odv[:, 0:2, :], osb[:, 0:2, :])
            bias_evac(nc.vector, 2, 0); bias_evac(nc.scalar, 2, 1)
            bias_evac(nc.vector, 3, 0); bias_evac(nc.scalar, 3, 1)
            nc.sync.dma_start(odv[:, 2:4, :], osb[:, 2:4, :])

    nc.compile()
    return nc


def kernel(X, Y, pi_dtw, classes):
    from concourse.bass_utils import run_bass_kernel_spmd

    X = np.asarray(X, dtype=np.float32)
    Y = np.asarray(Y, dtype=np.float32)
    pi_dtw = np.asarray(pi_dtw, dtype=np.float32)
    classes = np.asarray(classes).astype(np.int64)

    if "nc" not in _cache:
        _cache["nc"] = _build()
    nc = _cache["nc"]

    X8 = X.astype(FP8)
    Y8 = Y.astype(FP8)
    pi8 = pi_dtw.astype(FP8)
    idx = [np.nonzero(classes == c)[0] for c in range(C)]
    assert max(len(i) for i in idx) <= CPC, "class count exceeds capacity"

    # yt per Y half: [pp, (pt, d), j]
    yts = []
    for h in range(H):
        yh = Y8[h * NYH : (h + 1) * NYH]          # [j, p, d]
        a = yh.reshape(NYH, 2, 128, D).transpose(2, 1, 3, 0)  # [pp, pt, d, j]
        yts.append(np.ascontiguousarray(a.reshape(128, KC * NYH)))

    in_maps = []
    for k in range(NCORES):
        g, h = k >> 1, k & 1
        c0, c1 = 2 * g, 2 * g + 1
        xg = np.zeros((CAP, T, D), dtype=FP8)
        xg[0 : len(idx[c0])] = X8[idx[c0]]
        xg[CPC : CPC + len(idx[c1])] = X8[idx[c1]]
        # xk: [tp, d, tt, n]
        a = xg.reshape(CAP, 2, 128, D).transpose(2, 3, 1, 0)
        xk = np.ascontiguousarray(a.reshape(128, KC * CAP))
        # pis: pi [tp, c, tt, p] ++ piT [pp, c, pt, t]
        pg = pi8[[c0, c1]]                         # [c, t, p]
        b = pg.reshape(2, 2, 128, TP).transpose(2, 0, 1, 3)          # [tp,c,tt,p]
        bt = pg.reshape(2, TP, 2, 128).transpose(3, 0, 2, 1)         # [pp,c,pt,t]
        pik = np.concatenate(
            [b.reshape(128, -1), bt.reshape(128, -1)], axis=1
        )
        in_maps.append({"pis": np.ascontiguousarray(pik), "xk": xk, "yt": yts[h]})

    res = run_bass_kernel_spmd(nc, in_maps, core_ids=list(range(NCORES)))

    out = np.empty((N, NY), dtype=np.float32)
    for k in range(NCORES):
        g, h = k >> 1, k & 1
        blk = np.asarray(res.results[k]["outp"]).astype(np.float32)  # [j, n]
        jsel = slice(h * NYH, (h + 1) * NYH)
        c0, c1 = 2 * g, 2 * g + 1
        out[idx[c0], jsel] = blk[:, 0 : len(idx[c0])].T
        out[idx[c1], jsel] = blk[:, CPC : CPC + len(idx[c1])].T
    return out


# revision 104
# speedup vs baseline: 1.0009x; 1.0009x over previous
"""Trainium2 Bass kernel for the CNN-MAD per-class DTW transport cost.

Math (reference):
  mat_cost[n, j] = C1[n] + C2[c_n, j] - 2*C3[n, j],  c_n = classes[n]
    C1[n]    = sum_t rowsum[c_n, t] * ||X[n,t,:]||^2
    C2[c, j] = sum_p colsum[c, p] * ||Y[j,p,:]||^2
    C3[n, j] = sum_{p,d} (sum_t pi[c_n,t,p] X[n,t,d]) * Y[j,p,d]

Sharding 4x2: core k = (g, h) with g = k>>1 (class group: classes 2g, 2g+1,
each padded to 144 sample slots) and h = k&1 (Y half, 512 rows). The host
only regroups / transposes / dtype-casts; all arithmetic is on device.

Precision: inputs cast to fp8 e4m3 on host (X, Y absmax ~5.4; pi is 0/1 so
exact). Heavy matmuls run fp8 DoubleRow (K=256/instr) into f32 PSUM. The
large C1/C2 terms ride fp16 paths (C1 via a K=1 ones matmul into PSUM, C2
transposed [j, c] added as a per-partition bias during the fp16 output
evacuation). End-to-end rel err ~1e-3 vs the 2e-2 gate.

Device layout per core (C3 contraction k=(pt,d,pp), t=(tt,tp)):
  pis [tp 128, (c 2, tt 2, p 256) | (c 2, pt 2, t 256)]  fp8 (pi and pi^T)
  xk  [tp 128, d 8, tt 2, n 288]  fp8   xk[tp,d,tt,n] = X[n, tt*128+tp, d]
  yt  [pp 128, kc 16, j 512]      fp8   kc=(pt,d): yt = Y[j, pt*128+pp, d]
  crps: colsum^T / rowsum^T via 8 DoubleRow ones-matmuls (one PSUM bank)
  XW:  per (pt,d) granule, 2 class DoubleRows; evac * -2 -> xwt fp8
  xsq/ysq: elementwise fp8 squares split across ACT/DVE/Pool
  C1:  DoubleRow rw8.T @ xsq -> psum [2, 288] -> fp16 + per-class select
       -> c1row [1, 288]; added into each C3 psum by a K=1 ones matmul
  C2t: DoubleRow ysq.T @ cs2 -> psum [j 128, c 2] per jt (transposed C2)
  C3:  kc-pair DoubleRow yt.T @ xwt into 4 psum banks [j 128, n 288]
  out: fp16 evac with per-partition bias C2t[j, c], two DMAs out
"""

import sys

sys.path.insert(0, "/opt/trn_rl_repo")

import numpy as np
import ml_dtypes

N, NY, T, TP, D, C = 1024, 1024, 256, 256, 8, 8
NCORES = 8
G, H = 4, 2          # class groups x Y halves
CPC = 144            # per-class sample capacity (max count is 144)
CAP = 2 * CPC        # 288 sample columns per core
NYH = NY // H        # 512
KC = 16              # 128-row contraction chunks of C3, kc = (pt, d)
JT = NYH // 128      # 4 output row tiles

FP8 = ml_dtypes.float8_e4m3

_cache = {}


def _build():
    import concourse.bacc as bacc
    import concourse.mybir as mybir
    import concourse.tile as tile

    f8 = mybir.dt.float8e4
    f16 = mybir.dt.float16
    f32 = mybir.dt.float32
    DR = mybir.MatmulPerfMode.DoubleRow
    Ident = mybir.ActivationFunctionType.Identity
    nc = bacc.Bacc("TRN2", target_bir_lowering=False, debug=False, num_devices=NCORES)

    pis_d = nc.dram_tensor("pis", [128, 2 * 2 * 2 * TP], f8, kind="ExternalInput")
    xk_d = nc.dram_tensor("xk", [128, KC * CAP], f8, kind="ExternalInput")
    yt_d = nc.dram_tensor("yt", [128, KC * NYH], f8, kind="ExternalInput")
    out_d = nc.dram_tensor("outp", [NYH, CAP], f16, kind="ExternalOutput")

    with tile.TileContext(nc) as tc:
        with (
            tc.tile_pool(name="const", bufs=1) as pc,
            tc.tile_pool(name="xin", bufs=1) as px,
            tc.tile_pool(name="yin", bufs=1) as py,
            tc.tile_pool(name="psA", bufs=7, space="PSUM") as psA,
            tc.tile_pool(name="psB", bufs=1, space="PSUM") as psB,
        ):
            # ---- input DMAs on the SP HWDGE queue ----
            pis = pc.tile([128, 2, 2, 2, TP], f8, tag="pis")
            pisv = pis_d.rearrange("l (w c u p) -> l w c u p", w=2, c=2, u=2)
            pi = pis[:, 0, :, :, :]    # [tp, c, tt, p]
            piT = pis[:, 1, :, :, :]   # [pp, c, pt, t]

            xk = px.tile([128, D, 2, CAP], f8, tag="xk")
            xkv = xk_d.rearrange("l (d u n) -> l d u n", d=D, u=2)
            yt = py.tile([128, KC, NYH], f8, tag="yt")
            ytv = yt_d.rearrange("l (k j) -> l k j", k=KC)
            # pi first alone (crps/XW need it); piT only feeds the late
            # rowsum -> C1 path, so it rides after xk half 1
            nc.sync.dma_start(pis[:, 0, :, :, :], pisv[:, 0, :, :, :])
            nc.sync.dma_start(yt[:, 0:4, :], ytv[:, 0:4, :])
            nc.sync.dma_start(yt[:, 4:8, :], ytv[:, 4:8, :])
            nc.sync.dma_start(xk[:, 0:4, :, :], xkv[:, 0:4, :, :])
            nc.sync.dma_start(pis[:, 1, :, :, :], pisv[:, 1, :, :, :])
            nc.sync.dma_start(yt[:, 8:12, :], ytv[:, 8:12, :])
            nc.sync.dma_start(xk[:, 4:8, :, :], xkv[:, 4:8, :, :])
            nc.sync.dma_start(yt[:, 12:16, :], ytv[:, 12:16, :])

            # ---- small constants (Pool; everything tiny and early) ----
            # DoubleRow operands need a k-group stride that is a multiple of
            # 16 bytes (s3_lw_dual_fp8_restrictions), hence the padded tiles.
            ones8p = pc.tile([128, 2, 16], f8, tag="ones8p")
            nc.gpsimd.memset(ones8p[:], 1.0)
            ones8 = ones8p[:, :, 0:1]
            ones16 = pc.tile([1, 128], f16, tag="ones16")
            nc.gpsimd.memset(ones16[:], 1.0)

            # ---- colsum^T (c,pt) and rowsum^T (c,tt) via ones DoubleRows ----
            # cols 0-7: colsum/rowsum sums; cols 8-295 host the C1 rows
            # later (the bank is fully consumed by cs2/rw8p by then, so
            # C1's start=True bank-wipe is harmless)
            crps = psB.tile([128, 16 + CAP], f32, tag="psB", name="crps")
            for c in range(2):
                for pt in range(2):
                    nc.tensor.matmul(
                        crps[:, 2 * c + pt : 2 * c + pt + 1],
                        pi[:, c, :, pt * 128 : (pt + 1) * 128],
                        ones8,
                        start=(c == 0 and pt == 0), stop=True, perf_mode=DR,
                        skip_group_check=True,
                    )
            for c in range(2):
                for tt in range(2):
                    nc.tensor.matmul(
                        crps[:, 4 + 2 * c + tt : 5 + 2 * c + tt],
                        piT[:, c, :, tt * 128 : (tt + 1) * 128],
                        ones8,
                        start=False, stop=True, perf_mode=DR,
                        skip_group_check=True,
                    )
            cs2 = pc.tile([128, 2, 2], f8, tag="cs2")      # [pp, pt, c]
            csv = crps[:, 0:4].rearrange("l (c pt) -> l pt c", c=2)
            nc.vector.tensor_copy(cs2[:], csv)
            rw8p = pc.tile([128, 2, 16], f8, tag="rw8p")   # [tp, tt, c pad]
            nc.vector.tensor_copy(rw8p[:, :, 0:2], crps[:, 4:8].rearrange("l (c u) -> l u c", c=2))

            # one xwt tile per kc PAIR, each written by a single engine:
            # C3's pair-r read then depends only on its own pair's evac
            # (a shared tile serialized all C3 behind the last evacuation)
            xwp = [
                px.tile([128, 2, CAP], f8, tag=f"xwp{i}", name=f"xwp{i}")
                for i in range(8)
            ]
            xsq = px.tile([128, D, 2, CAP], f8, tag="xsq")
            # one ysq tile per writer engine: a shared tile serializes the
            # writers through false whole-tile WAW dependencies
            ysqD = py.tile([128, 2, NYH], f8, tag="ysqD")    # kc 0-1   (DVE)
            ysqB = py.tile([128, 2, NYH], f8, tag="ysqB")    # kc 2-3   (ACT)
            ysqP = py.tile([128, 8, NYH], f8, tag="ysqP")    # kc 4-11  (Pool)
            ysqE = py.tile([128, 2, NYH], f8, tag="ysqE")    # kc 12-13 (DVE)
            ysqA = py.tile([128, 2, NYH], f8, tag="ysqA")    # kc 14-15 (ACT)

            def ysq_sl(kc, js):
                if kc < 2:
                    return ysqD[:, kc, js]
                if kc < 4:
                    return ysqB[:, kc - 2, js]
                if kc < 12:
                    return ysqP[:, kc - 4, js]
                if kc < 14:
                    return ysqE[:, kc - 12, js]
                return ysqA[:, kc - 14, js]

            def xw_granule(pt, d, evac):
                g = psA.tile([128, CAP], f32, tag="psA", name=f"xw{pt}_{d}")
                for cl in range(2):
                    nc.tensor.matmul(
                        g[:, cl * CPC : (cl + 1) * CPC],
                        pi[:, cl, :, pt * 128 : (pt + 1) * 128],
                        xk[:, d, :, cl * CPC : (cl + 1) * CPC],
                        start=True, stop=True, perf_mode=DR,
                        skip_group_check=True,
                    )
                evac(xwp[pt * 4 + d // 2][:, d % 2, :], g[:], -2.0)

            dve_m, act_m, pool_m = (nc.vector.tensor_scalar_mul, nc.scalar.mul,
                                    nc.gpsimd.tensor_scalar_mul)

            # Pool (GPSIMD) cannot touch PSUM on real HW, and only plain
            # tensor_tensor ops compile for it: it streams mid ysq chunks.
            def pool_sq(dst, src):
                nc.gpsimd.tensor_mul(dst, src, src)

            # early ysq on the fast engines while they wait for xk; Pool
            # streams the mid chunks kc 4-11
            nc.vector.tensor_mul(ysqD[:], yt[:, 0:2, :], yt[:, 0:2, :])
            nc.scalar.square(ysqB[:], yt[:, 2:4, :])
            pool_sq(ysqP[:, 0:4, :], yt[:, 4:8, :])
            pool_sq(ysqP[:, 4:8, :], yt[:, 8:12, :])

            # ---- xk half 1 (d 0-3): XW kc 0-3, 8-11; xsq quad on DVE ----
            # ACT has exec-queue depth 0 (strictly in-order), so its queue
            # must be emitted in data-arrival order: the (1, d2-3) granule
            # evacs (ready late) go AFTER xsq_b below, or they block it.
            for d in range(4):
                xw_granule(0, d, (dve_m, dve_m, act_m, act_m)[d])
            for d in range(2):
                xw_granule(1, d, dve_m)
            nc.vector.tensor_mul(xsq[:, 0:2, :, :], xk[:, 0:2, :, :], xk[:, 0:2, :, :])
            nc.vector.tensor_mul(xsq[:, 2:4, :, :], xk[:, 2:4, :, :], xk[:, 2:4, :, :])

            # ---- xk half 2 (d 4-7): xsq quad on ACT first (C1 chain) ----
            nc.scalar.square(xsq[:, 4:8, :, :], xk[:, 4:8, :, :])
            xw_granule(1, 2, act_m)
            xw_granule(1, 3, act_m)
            # final ysq chunks slot in here: the data lands about when the
            # engines reach this point, and C2 can close early
            nc.vector.tensor_mul(ysqE[:, 0:1, :], yt[:, 12:13, :], yt[:, 12:13, :])
            nc.vector.tensor_mul(ysqE[:, 1:2, :], yt[:, 13:14, :], yt[:, 13:14, :])
            nc.scalar.square(ysqA[:], yt[:, 14:16, :])
            for d in range(4, 8):
                xw_granule(0, d, (dve_m, dve_m, act_m, act_m)[d - 4])
            for d in range(4, 8):
                xw_granule(1, d, (dve_m, dve_m, act_m, act_m)[d - 4])

            # ---- C1 per class: DoubleRows -> two [1, CAP] psum rows ----
            # Each class's C1 only needs its own 144 columns, so both rows
            # fit disjoint regions of the crps bank (no psA slot: the psA
            # rotation edge made C1 gate the augs). One start/stop bracket
            # for the shared bank, one copy out.
            for d in range(D):
                for cl in range(2):
                    nc.tensor.matmul(
                        crps[0:1, 8 + cl * CPC : 8 + (cl + 1) * CPC],
                        rw8p[:, :, cl : cl + 1],
                        xsq[:, d, :, cl * CPC : (cl + 1) * CPC],
                        start=False,
                        stop=(d == D - 1 and cl == 1),
                        perf_mode=DR, skip_group_check=True,
                    )
            c1row = pc.tile([1, CAP], f16, tag="c1row")
            nc.vector.tensor_copy(c1row[0:1, :], crps[0:1, 8 : 8 + CAP])


            # ---- streaming: remaining ysq + C3/C2t per kc pair ----
            c2ps = crps[:, 8 + CAP : 16 + CAP].rearrange(
                "l (jt c) -> l jt c", c=2
            )
            c3ps = [
                psA.tile([128, CAP], f32, tag="psA", name=f"c3_{jt}")
                for jt in range(JT)
            ]

            def c3_sl(jt):
                return c3ps[jt][:]

            def c3_r(r, start):
                for jt in range(JT):
                    nc.tensor.matmul(
                        c3_sl(jt),
                        yt[:, 2 * r : 2 * r + 2, jt * 128 : (jt + 1) * 128],
                        xwp[r][:],
                        start=start, stop=False, perf_mode=DR,
                        skip_group_check=True,
                    )

            def c2_r(r, start, stop):
                # Plain matmuls (2-wide-ifmap DoubleRow miscomputes on HW).
                # One start/stop bracket for the whole bank: a start=True
                # clears has-written for the entire 2KB PSUM bank, so only
                # the very first matmul of the shared bank may carry it.
                pt = r // 4
                for kc in (2 * r, 2 * r + 1):
                    for jt in range(JT):
                        nc.tensor.matmul(
                            c2ps[:, jt, :],
                            ysq_sl(kc, slice(jt * 128, (jt + 1) * 128)),
                            cs2[:, pt, :],
                            start=False,
                            stop=(stop and kc == 2 * r + 1 and jt == JT - 1),
                            skip_group_check=True,
                        )

            c3_r(0, True); c2_r(0, True, False)
            c3_r(1, False); c2_r(1, False, False)
            c3_r(2, False); c2_r(2, False, False)
            c3_r(3, False); c2_r(3, False, False)
            c3_r(4, False); c2_r(4, False, False)
            c3_r(5, False); c2_r(5, False, False)
            c3_r(6, False); c2_r(6, False, False)
            # close C2 first: it only needs ysq, so the C2 bias is ready
            # long before the C3 psums close
            c2_r(7, False, True)
            c2sb = pc.tile([128, JT, 2], f32, tag="c2sb")
            nc.vector.tensor_copy(c2sb[:], c2ps[:])
            # close each C3 group with one K=1 fp16 C1 augmentation right
            # after its last kc pair
            for jt in range(JT):
                nc.tensor.matmul(
                    c3_sl(jt),
                    yt[:, 14:16, jt * 128 : (jt + 1) * 128],
                    xwp[7][:],
                    start=False, stop=False, perf_mode=DR,
                    skip_group_check=True,
                )
                nc.tensor.matmul(
                    c3_sl(jt), ones16[:], c1row[:],
                    start=False, stop=True, skip_group_check=True,
                )

            # ---- out: fp16 evac with per-partition C2 bias, 2 DMAs ----
            osb = py.tile([128, JT, CAP], f16, tag="osb")
            odv = out_d.rearrange("(jt l) n -> l jt n", l=128)

            def bias_evac(eng, jt, cl):
                dst = osb[:, jt, cl * CPC : (cl + 1) * CPC]
                src = c3_sl(jt)[:, cl * CPC : (cl + 1) * CPC]
                b = c2sb[:, jt, cl : cl + 1]
                if eng is nc.scalar:
                    eng.activation(dst, src, Ident, bias=b)
                else:
                    eng.tensor_scalar_add(dst, src, b)

            bias_evac(nc.vector, 0, 0); bias_evac(nc.scalar, 0, 1)
            bias_evac(nc.vector, 1, 0); bias_evac(nc.scalar, 1, 1)
            nc.sync.dma_start(odv[:, 0:2, :], osb[:, 0:2, :])
            bias_evac(nc.vector, 2, 0); bias_evac(nc.scalar, 2, 1)
            bias_evac(nc.vector, 3, 0); bias_evac(nc.scalar, 3, 1)
            nc.sync.dma_start(odv[:, 2:4, :], osb[:, 2:4, :])

    nc.compile()
    return nc


def kernel(X, Y, pi_dtw, classes):
    from concourse.bass_utils import run_bass_kernel_spmd

    X = np.asarray(X, dtype=np.float32)
    Y = np.asarray(Y, dtype=np.float32)
    pi_dtw = np.asarray(pi_dtw, dtype=np.float32)
    classes = np.asarray(classes).astype(np.int64)

    if "nc" not in _cache:
        _cache["nc"] = _build()
    nc = _cache["nc"]

    X8 = X.astype(FP8)
    Y8 = Y.astype(FP8)
    pi8 = pi_dtw.astype(FP8)
    idx = [np.nonzero(classes == c)[0] for c in range(C)]
    assert max(len(i) for i in idx) <= CPC, "class count exceeds capacity"

    # yt per Y half: [pp, (pt, d), j]
    yts = []
    for h in range(H):
        yh = Y8[h * NYH : (h + 1) * NYH]          # [j, p, d]
        a = yh.reshape(NYH, 2, 128, D).transpose(2, 1, 3, 0)  # [pp, pt, d, j]
        yts.append(np.ascontiguousarray(a.reshape(128, KC * NYH)))

    in_maps = []
    for k in range(NCORES):
        g, h = k >> 1, k & 1
        c0, c1 = 2 * g, 2 * g + 1
        xg = np.zeros((CAP, T, D), dtype=FP8)
        xg[0 : len(idx[c0])] = X8[idx[c0]]
        xg[CPC : CPC + len(idx[c1])] = X8[idx[c1]]
        # xk: [tp, d, tt, n]
        a = xg.reshape(CAP, 2, 128, D).transpose(2, 3, 1, 0)
        xk = np.ascontiguousarray(a.reshape(128, KC * CAP))
        # pis: pi [tp, c, tt, p] ++ piT [pp, c, pt, t]
        pg = pi8[[c0, c1]]                         # [c, t, p]
        b = pg.reshape(2, 2, 128, TP).transpose(2, 0, 1, 3)          # [tp,c,tt,p]
        bt = pg.reshape(2, TP, 2, 128).transpose(3, 0, 2, 1)         # [pp,c,pt,t]
        pik = np.concatenate(
            [b.reshape(128, -1), bt.reshape(128, -1)], axis=1
        )
        in_maps.append({"pis": np.ascontiguousarray(pik), "xk": xk, "yt": yts[h]})

    res = run_bass_kernel_spmd(nc, in_maps, core_ids=list(range(NCORES)))

    out = np.empty((N, NY), dtype=np.float32)
    for k in range(NCORES):
        g, h = k >> 1, k & 1
        blk = np.asarray(res.results[k]["outp"]).astype(np.float32)  # [j, n]
        jsel = slice(h * NYH, (h + 1) * NYH)
        c0, c1 = 2 * g, 2 * g + 1
        out[idx[c0], jsel] = blk[:, 0 : len(idx[c0])].T
        out[idx[c1], jsel] = blk[:, CPC : CPC + len(idx[c1])].T
    return out


# revision 105
# speedup vs baseline: 1.0077x; 1.0068x over previous
"""Trainium2 Bass kernel for the CNN-MAD per-class DTW transport cost.

Math (reference):
  mat_cost[n, j] = C1[n] + C2[c_n, j] - 2*C3[n, j],  c_n = classes[n]
    C1[n]    = sum_t rowsum[c_n, t] * ||X[n,t,:]||^2
    C2[c, j] = sum_p colsum[c, p] * ||Y[j,p,:]||^2
    C3[n, j] = sum_{p,d} (sum_t pi[c_n,t,p] X[n,t,d]) * Y[j,p,d]

Sharding 4x2: core k = (g, h) with g = k>>1 (class group: classes 2g, 2g+1,
each padded to 144 sample slots) and h = k&1 (Y half, 512 rows). The host
only regroups / transposes / dtype-casts; all arithmetic is on device.

Precision: inputs cast to fp8 e4m3 on host (X, Y absmax ~5.4; pi is 0/1 so
exact). Heavy matmuls run fp8 DoubleRow (K=256/instr) into f32 PSUM. The
large C1/C2 terms ride fp16 paths (C1 via a K=1 ones matmul into PSUM, C2
transposed [j, c] added as a per-partition bias during the fp16 output
evacuation). End-to-end rel err ~1e-3 vs the 2e-2 gate.

Device layout per core (C3 contraction k=(pt,d,pp), t=(tt,tp)):
  pis [tp 128, (c 2, tt 2, p 256) | (c 2, pt 2, t 256)]  fp8 (pi and pi^T)
  xk  [tp 128, d 8, tt 2, n 288]  fp8   xk[tp,d,tt,n] = X[n, tt*128+tp, d]
  yt  [pp 128, kc 16, j 512]      fp8   kc=(pt,d): yt = Y[j, pt*128+pp, d]
  crps: colsum^T / rowsum^T via 8 DoubleRow ones-matmuls (one PSUM bank)
  XW:  per (pt,d) granule, 2 class DoubleRows; evac * -2 -> xwt fp8
  xsq/ysq: elementwise fp8 squares split across ACT/DVE/Pool
  C1:  DoubleRow rw8.T @ xsq -> psum [2, 288] -> fp16 + per-class select
       -> c1row [1, 288]; added into each C3 psum by a K=1 ones matmul
  C2t: DoubleRow ysq.T @ cs2 -> psum [j 128, c 2] per jt (transposed C2)
  C3:  kc-pair DoubleRow yt.T @ xwt into 4 psum banks [j 128, n 288]
  out: fp16 evac with per-partition bias C2t[j, c], two DMAs out
"""

import sys

sys.path.insert(0, "/opt/trn_rl_repo")

import numpy as np
import ml_dtypes

N, NY, T, TP, D, C = 1024, 1024, 256, 256, 8, 8
NCORES = 8
G, H = 4, 2          # class groups x Y halves
CPC = 144            # per-class sample capacity (max count is 144)
CAP = 2 * CPC        # 288 sample columns per core
NYH = NY // H        # 512
KC = 16              # 128-row contraction chunks of C3, kc = (pt, d)
JT = NYH // 128      # 4 output row tiles

FP8 = ml_dtypes.float8_e4m3

_cache = {}


def _build():
    import concourse.bacc as bacc
    import concourse.mybir as mybir
    import concourse.tile as tile

    f8 = mybir.dt.float8e4
    f16 = mybir.dt.float16
    f32 = mybir.dt.float32
    DR = mybir.MatmulPerfMode.DoubleRow
    Ident = mybir.ActivationFunctionType.Identity
    nc = bacc.Bacc("TRN2", target_bir_lowering=False, debug=False, num_devices=NCORES)

    pis_d = nc.dram_tensor("pis", [128, 2 * 2 * 2 * TP], f8, kind="ExternalInput")
    xk_d = nc.dram_tensor("xk", [128, KC * CAP], f8, kind="ExternalInput")
    yt_d = nc.dram_tensor("yt", [128, KC * NYH], f8, kind="ExternalInput")
    out_d = nc.dram_tensor("outp", [NYH, CAP], f16, kind="ExternalOutput")

    with tile.TileContext(nc) as tc:
        with (
            tc.tile_pool(name="const", bufs=1) as pc,
            tc.tile_pool(name="xin", bufs=1) as px,
            tc.tile_pool(name="yin", bufs=1) as py,
            tc.tile_pool(name="psA", bufs=7, space="PSUM") as psA,
            tc.tile_pool(name="psB", bufs=1, space="PSUM") as psB,
        ):
            # ---- input DMAs on the SP HWDGE queue ----
            pis = pc.tile([128, 2, 2, 2, TP], f8, tag="pis")
            pisv = pis_d.rearrange("l (w c u p) -> l w c u p", w=2, c=2, u=2)
            pi = pis[:, 0, :, :, :]    # [tp, c, tt, p]
            piT = pis[:, 1, :, :, :]   # [pp, c, pt, t]

            xk = px.tile([128, D, 2, CAP], f8, tag="xk")
            xkv = xk_d.rearrange("l (d u n) -> l d u n", d=D, u=2)
            yt = py.tile([128, KC, NYH], f8, tag="yt")
            ytv = yt_d.rearrange("l (k j) -> l k j", k=KC)
            # pi first alone (crps/XW need it); piT only feeds the late
            # rowsum -> C1 path, so it rides after xk half 1
            nc.sync.dma_start(pis[:, 0, :, :, :], pisv[:, 0, :, :, :])
            nc.sync.dma_start(yt[:, 0:4, :], ytv[:, 0:4, :])
            nc.sync.dma_start(yt[:, 4:8, :], ytv[:, 4:8, :])
            nc.sync.dma_start(xk[:, 0:4, :, :], xkv[:, 0:4, :, :])
            nc.sync.dma_start(pis[:, 1, :, :, :], pisv[:, 1, :, :, :])
            nc.sync.dma_start(yt[:, 8:12, :], ytv[:, 8:12, :])
            nc.sync.dma_start(xk[:, 4:8, :, :], xkv[:, 4:8, :, :])
            nc.sync.dma_start(yt[:, 12:16, :], ytv[:, 12:16, :])

            # ---- small constants (Pool; everything tiny and early) ----
            # DoubleRow operands need a k-group stride that is a multiple of
            # 16 bytes (s3_lw_dual_fp8_restrictions), hence the padded tiles.
            ones8p = pc.tile([128, 2, 16], f8, tag="ones8p")
            nc.gpsimd.memset(ones8p[:], 1.0)
            ones8 = ones8p[:, :, 0:1]
            ones16 = pc.tile([1, 128], f16, tag="ones16")
            nc.gpsimd.memset(ones16[:], 1.0)

            # ---- colsum^T (c,pt) and rowsum^T (c,tt) via ones DoubleRows ----
            # cols 0-7: colsum/rowsum sums; cols 8-295 host the C1 rows
            # later (the bank is fully consumed by cs2/rw8p by then, so
            # C1's start=True bank-wipe is harmless)
            crps = psB.tile([128, 16 + CAP], f32, tag="psB", name="crps")
            for c in range(2):
                for pt in range(2):
                    nc.tensor.matmul(
                        crps[:, 2 * c + pt : 2 * c + pt + 1],
                        pi[:, c, :, pt * 128 : (pt + 1) * 128],
                        ones8,
                        start=(c == 0 and pt == 0), stop=True, perf_mode=DR,
                        skip_group_check=True,
                    )
            for c in range(2):
                for tt in range(2):
                    nc.tensor.matmul(
                        crps[:, 4 + 2 * c + tt : 5 + 2 * c + tt],
                        piT[:, c, :, tt * 128 : (tt + 1) * 128],
                        ones8,
                        start=False, stop=True, perf_mode=DR,
                        skip_group_check=True,
                    )
            cs2 = pc.tile([128, 2, 2], f8, tag="cs2")      # [pp, pt, c]
            csv = crps[:, 0:4].rearrange("l (c pt) -> l pt c", c=2)
            nc.vector.tensor_copy(cs2[:], csv)
            rw8p = pc.tile([128, 2, 16], f8, tag="rw8p")   # [tp, tt, c pad]
            nc.vector.tensor_copy(rw8p[:, :, 0:2], crps[:, 4:8].rearrange("l (c u) -> l u c", c=2))

            # one xwt tile per kc PAIR, each written by a single engine:
            # C3's pair-r read then depends only on its own pair's evac
            # (a shared tile serialized all C3 behind the last evacuation)
            xwp = [
                px.tile([128, 2, CAP], f8, tag=f"xwp{i}", name=f"xwp{i}")
                for i in range(8)
            ]
            xsq = px.tile([128, D, 2, CAP], f8, tag="xsq")
            # one ysq tile per writer engine: a shared tile serializes the
            # writers through false whole-tile WAW dependencies
            ysqD = py.tile([128, 2, NYH], f8, tag="ysqD")    # kc 0-1   (DVE)
            ysqB = py.tile([128, 2, NYH], f8, tag="ysqB")    # kc 2-3   (ACT)
            ysqP = py.tile([128, 8, NYH], f8, tag="ysqP")    # kc 4-11  (Pool)
            ysqE = py.tile([128, 2, NYH], f8, tag="ysqE")    # kc 12-13 (DVE)
            ysqA = py.tile([128, 2, NYH], f8, tag="ysqA")    # kc 14-15 (ACT)

            def ysq_sl(kc, js):
                if kc < 2:
                    return ysqD[:, kc, js]
                if kc < 4:
                    return ysqB[:, kc - 2, js]
                if kc < 12:
                    return ysqP[:, kc - 4, js]
                if kc < 14:
                    return ysqE[:, kc - 12, js]
                return ysqA[:, kc - 14, js]

            def xw_granule(pt, d, evac):
                g = psA.tile([128, CAP], f32, tag="psA", name=f"xw{pt}_{d}")
                for cl in range(2):
                    nc.tensor.matmul(
                        g[:, cl * CPC : (cl + 1) * CPC],
                        pi[:, cl, :, pt * 128 : (pt + 1) * 128],
                        xk[:, d, :, cl * CPC : (cl + 1) * CPC],
                        start=True, stop=True, perf_mode=DR,
                        skip_group_check=True,
                    )
                evac(xwp[pt * 4 + d // 2][:, d % 2, :], g[:], -2.0)

            dve_m, act_m, pool_m = (nc.vector.tensor_scalar_mul, nc.scalar.mul,
                                    nc.gpsimd.tensor_scalar_mul)

            # Pool (GPSIMD) cannot touch PSUM on real HW, and only plain
            # tensor_tensor ops compile for it: it streams mid ysq chunks.
            def pool_sq(dst, src):
                nc.gpsimd.tensor_mul(dst, src, src)

            # early ysq on the fast engines while they wait for xk; Pool
            # streams the mid chunks kc 4-11
            nc.vector.tensor_mul(ysqD[:], yt[:, 0:2, :], yt[:, 0:2, :])
            nc.scalar.square(ysqB[:], yt[:, 2:4, :])
            pool_sq(ysqP[:, 0:4, :], yt[:, 4:8, :])
            pool_sq(ysqP[:, 4:8, :], yt[:, 8:12, :])

            # ---- xk half 1 (d 0-3): XW kc 0-3, 8-11; xsq quad on DVE ----
            # ACT has exec-queue depth 0 (strictly in-order), so its queue
            # must be emitted in data-arrival order: the (1, d2-3) granule
            # evacs (ready late) go AFTER xsq_b below, or they block it.
            for d in range(4):
                xw_granule(0, d, (dve_m, dve_m, act_m, act_m)[d])
            for d in range(2):
                xw_granule(1, d, dve_m)
            nc.vector.tensor_mul(xsq[:, 0:2, :, :], xk[:, 0:2, :, :], xk[:, 0:2, :, :])
            nc.vector.tensor_mul(xsq[:, 2:4, :, :], xk[:, 2:4, :, :], xk[:, 2:4, :, :])

            # ---- xk half 2 (d 4-7): xsq quad on ACT first (C1 chain) ----
            nc.scalar.square(xsq[:, 4:8, :, :], xk[:, 4:8, :, :])
            xw_granule(1, 2, act_m)
            xw_granule(1, 3, act_m)
            # final ysq chunks slot in here: the data lands about when the
            # engines reach this point, and C2 can close early
            nc.vector.tensor_mul(ysqE[:, 0:1, :], yt[:, 12:13, :], yt[:, 12:13, :])
            nc.vector.tensor_mul(ysqE[:, 1:2, :], yt[:, 13:14, :], yt[:, 13:14, :])
            nc.scalar.square(ysqA[:], yt[:, 14:16, :])
            for d in range(4, 8):
                xw_granule(0, d, (dve_m, dve_m, act_m, act_m)[d - 4])
            for d in range(4, 8):
                xw_granule(1, d, (dve_m, dve_m, act_m, act_m)[d - 4])

            # ---- C1 per class: DoubleRows -> two [1, CAP] psum rows ----
            # Each class's C1 only needs its own 144 columns, so both rows
            # fit disjoint regions of the crps bank (no psA slot: the psA
            # rotation edge made C1 gate the augs). One start/stop bracket
            # for the shared bank, one copy out.
            for d in range(D):
                for cl in range(2):
                    nc.tensor.matmul(
                        crps[0:1, 8 + cl * CPC : 8 + (cl + 1) * CPC],
                        rw8p[:, :, cl : cl + 1],
                        xsq[:, d, :, cl * CPC : (cl + 1) * CPC],
                        start=False,
                        stop=(d == D - 1 and cl == 1),
                        perf_mode=DR, skip_group_check=True,
                    )
            c1row = pc.tile([1, CAP], f16, tag="c1row")
            nc.vector.tensor_copy(c1row[0:1, :], crps[0:1, 8 : 8 + CAP])


            # ---- streaming: remaining ysq + C3/C2t per kc pair ----
            c2ps = crps[:, 8 + CAP : 16 + CAP].rearrange(
                "l (jt c) -> l jt c", c=2
            )
            c3ps = [
                psA.tile([128, CAP], f32, tag="psA", name=f"c3_{jt}")
                for jt in range(JT)
            ]

            def c3_sl(jt):
                return c3ps[jt][:]

            def c3_r(r, start):
                for jt in range(JT):
                    nc.tensor.matmul(
                        c3_sl(jt),
                        yt[:, 2 * r : 2 * r + 2, jt * 128 : (jt + 1) * 128],
                        xwp[r][:],
                        start=start, stop=False, perf_mode=DR,
                        skip_group_check=True,
                    )

            def c2_r(r, start, stop):
                # Plain matmuls (2-wide-ifmap DoubleRow miscomputes on HW).
                # One start/stop bracket for the whole bank: a start=True
                # clears has-written for the entire 2KB PSUM bank, so only
                # the very first matmul of the shared bank may carry it.
                pt = r // 4
                for kc in (2 * r, 2 * r + 1):
                    for jt in range(JT):
                        nc.tensor.matmul(
                            c2ps[:, jt, :],
                            ysq_sl(kc, slice(jt * 128, (jt + 1) * 128)),
                            cs2[:, pt, :],
                            start=False,
                            stop=(stop and kc == 2 * r + 1 and jt == JT - 1),
                            skip_group_check=True,
                        )

            c3_r(0, True); c2_r(0, True, False)
            c3_r(1, False); c2_r(1, False, False)
            c3_r(2, False); c2_r(2, False, False)
            c3_r(3, False); c2_r(3, False, False)
            c3_r(4, False); c2_r(4, False, False)
            c3_r(5, False); c2_r(5, False, False)
            # C2's last pairs run before c3_r(6): their ysq inputs are
            # ready well before c3_r(6)'s xwt pair, so C2 closes early
            c2_r(6, False, False)
            c2_r(7, False, True)
            c3_r(6, False)
            c2sb = pc.tile([128, JT, 2], f32, tag="c2sb")
            nc.vector.tensor_copy(c2sb[:], c2ps[:])
            # close each C3 group with one K=1 fp16 C1 augmentation right
            # after its last kc pair
            for jt in range(JT):
                nc.tensor.matmul(
                    c3_sl(jt),
                    yt[:, 14:16, jt * 128 : (jt + 1) * 128],
                    xwp[7][:],
                    start=False, stop=False, perf_mode=DR,
                    skip_group_check=True,
                )
                nc.tensor.matmul(
                    c3_sl(jt), ones16[:], c1row[:],
                    start=False, stop=True, skip_group_check=True,
                )

            # ---- out: fp16 evac with per-partition C2 bias, 2 DMAs ----
            osb = py.tile([128, JT, CAP], f16, tag="osb")
            odv = out_d.rearrange("(jt l) n -> l jt n", l=128)

            def bias_evac(eng, jt, cl):
                dst = osb[:, jt, cl * CPC : (cl + 1) * CPC]
                src = c3_sl(jt)[:, cl * CPC : (cl + 1) * CPC]
                b = c2sb[:, jt, cl : cl + 1]
                if eng is nc.scalar:
                    eng.activation(dst, src, Ident, bias=b)
                else:
                    eng.tensor_scalar_add(dst, src, b)

            bias_evac(nc.vector, 0, 0); bias_evac(nc.scalar, 0, 1)
            bias_evac(nc.vector, 1, 0); bias_evac(nc.scalar, 1, 1)
            nc.sync.dma_start(odv[:, 0:2, :], osb[:, 0:2, :])
            bias_evac(nc.vector, 2, 0); bias_evac(nc.scalar, 2, 1)
            bias_evac(nc.vector, 3, 0); bias_evac(nc.scalar, 3, 1)
            nc.sync.dma_start(odv[:, 2:4, :], osb[:, 2:4, :])

    nc.compile()
    return nc


def kernel(X, Y, pi_dtw, classes):
    from concourse.bass_utils import run_bass_kernel_spmd

    X = np.asarray(X, dtype=np.float32)
    Y = np.asarray(Y, dtype=np.float32)
    pi_dtw = np.asarray(pi_dtw, dtype=np.float32)
    classes = np.asarray(classes).astype(np.int64)

    if "nc" not in _cache:
        _cache["nc"] = _build()
    nc = _cache["nc"]

    X8 = X.astype(FP8)
    Y8 = Y.astype(FP8)
    pi8 = pi_dtw.astype(FP8)
    idx = [np.nonzero(classes == c)[0] for c in range(C)]
    assert max(len(i) for i in idx) <= CPC, "class count exceeds capacity"

    # yt per Y half: [pp, (pt, d), j]
    yts = []
    for h in range(H):
        yh = Y8[h * NYH : (h + 1) * NYH]          # [j, p, d]
        a = yh.reshape(NYH, 2, 128, D).transpose(2, 1, 3, 0)  # [pp, pt, d, j]
        yts.append(np.ascontiguousarray(a.reshape(128, KC * NYH)))

    in_maps = []
    for k in range(NCORES):
        g, h = k >> 1, k & 1
        c0, c1 = 2 * g, 2 * g + 1
        xg = np.zeros((CAP, T, D), dtype=FP8)
        xg[0 : len(idx[c0])] = X8[idx[c0]]
        xg[CPC : CPC + len(idx[c1])] = X8[idx[c1]]
        # xk: [tp, d, tt, n]
        a = xg.reshape(CAP, 2, 128, D).transpose(2, 3, 1, 0)
        xk = np.ascontiguousarray(a.reshape(128, KC * CAP))
        # pis: pi [tp, c, tt, p] ++ piT [pp, c, pt, t]
        pg = pi8[[c0, c1]]                         # [c, t, p]
        b = pg.reshape(2, 2, 128, TP).transpose(2, 0, 1, 3)          # [tp,c,tt,p]
        bt = pg.reshape(2, TP, 2, 128).transpose(3, 0, 2, 1)         # [pp,c,pt,t]
        pik = np.concatenate(
            [b.reshape(128, -1), bt.reshape(128, -1)], axis=1
        )
        in_maps.append({"pis": np.ascontiguousarray(pik), "xk": xk, "yt": yts[h]})

    res = run_bass_kernel_spmd(nc, in_maps, core_ids=list(range(NCORES)))

    out = np.empty((N, NY), dtype=np.float32)
    for k in range(NCORES):
        g, h = k >> 1, k & 1
        blk = np.asarray(res.results[k]["outp"]).astype(np.float32)  # [j, n]
        jsel = slice(h * NYH, (h + 1) * NYH)
        c0, c1 = 2 * g, 2 * g + 1
        out[idx[c0], jsel] = blk[:, 0 : len(idx[c0])].T
        out[idx[c1], jsel] = blk[:, CPC : CPC + len(idx[c1])].T
    return out
